# revision 36
# baseline (speedup 1.0000x reference)
"""Trainium2 Bass kernel for nn_Decoder8 (gnn_message_passing).

Contract: kernel(**inputs) takes FULL unsharded inputs (np arrays, keyed as in
setup_inputs()), returns (logits [32,42] f32, structure_feature [32,512,300] f32).

Strategy: pure data parallelism -- batch 32 split 4 elems/core x 8 cores,
parameters replicated. All activations kept feature-major [feat, 512] on chip;
host passes inputs pre-transposed, adjacencies transposed as uint8, embedding
rows pre-gathered, weights packed/padded/folded.
"""

import numpy as np

import concourse.bacc as bacc
import concourse.tile as tile
import concourse.bass as bass
from concourse import mybir, bass_utils

F32 = mybir.dt.float32
F32R = mybir.dt.float32r
U8 = mybir.dt.uint8

BS, SEQ, HID = 32, 512, 768
NCORES = 8
EPC = BS // NCORES           # elems per core
POS_IND = 300
NEG = -1e9
ATTN_DIM = 64
HIDDEN = 300
FF = 600
LABEL_DIM = 200
N_LABELS = 42
P = 128


def _pad_rows(w, rows):
    out = np.zeros((rows, w.shape[1]), np.float32)
    out[: w.shape[0]] = w
    return out


def _pack_bias(b, ntiles):
    out = np.zeros((ntiles, P), np.float32)
    out.reshape(-1)[: b.shape[0]] = b
    return out  # [ntiles, P]; DMA'd then viewed [P, ntiles]


def _prep_params(params):
    """Pack / pad / fold all weights host-side. Returns dict name->np.ndarray."""
    g = {}
    dga = params["dga"]
    for li, lp in enumerate(dga):
        din = 828 if li == 0 else HIDDEN
        kpad = 896 if li == 0 else 384
        # column order [qf qb | kf kb | vf vb] each 64 wide
        w = np.concatenate(
            [lp[n]["w"] for n in ("q_f", "q_b", "k_f", "k_b", "v_f", "v_b")], axis=1
        )  # [din, 384]
        g[f"wqkv{li}"] = _pad_rows(np.asarray(w, np.float32), kpad)
        b = np.concatenate(
            [np.asarray(lp[n]["b"], np.float32) for n in ("q_f", "q_b", "k_f", "k_b", "v_f", "v_b")]
        )  # [384]
        scale = np.ones(384, np.float32)
        scale[0:128] = 0.125     # q_f | q_b
        g[f"bqkv{li}"] = _pack_bias(b * scale, 3)
        g[f"sqkv{li}"] = _pack_bias(scale, 3)
        g[f"wo{li}"] = np.asarray(lp["o"]["w"], np.float32)          # [128,300]
        g[f"bo{li}"] = _pack_bias(np.asarray(lp["o"]["b"], np.float32), 3)
        g[f"wff1_{li}"] = _pad_rows(np.asarray(lp["ff1"]["w"], np.float32), 384)
        g[f"bff1_{li}"] = _pack_bias(np.asarray(lp["ff1"]["b"], np.float32), 5)
        g[f"wff2_{li}"] = _pad_rows(np.asarray(lp["ff2"]["w"], np.float32), 640)
        g[f"bff2_{li}"] = _pack_bias(np.asarray(lp["ff2"]["b"], np.float32), 3)

    label_emb = np.asarray(params["label_emb"], np.float32)          # [42,200]
    w_lak = np.asarray(params["la_k"]["w"], np.float32)              # [200,256]
    b_lak = np.asarray(params["la_k"]["b"], np.float32)              # [256]
    ktsum = (label_emb[1:] @ w_lak + b_lak).sum(0)                   # [256]
    w_laq = np.asarray(params["la_q"]["w"], np.float32)              # [300,256]
    v_la = (w_laq @ ktsum) * 0.125                                   # [300]
    g["v_la"] = _pad_rows(v_la[:, None], 384)                        # [384,1]

    wfeat = np.asarray(params["feat"]["w"], np.float32)              # [900,300]
    bfeat = np.asarray(params["feat"]["b"], np.float32)
    wfs, wfo, wfc = wfeat[0:300], wfeat[300:600], wfeat[600:900]
    wsub = np.asarray(params["subj_mlp"]["w"], np.float32)           # [768,300]
    bsub = np.asarray(params["subj_mlp"]["b"], np.float32)
    wobj = np.asarray(params["obj_mlp"]["w"], np.float32)
    bobj = np.asarray(params["obj_mlp"]["b"], np.float32)
    g["wfs"] = wsub @ wfs                                            # [768,300]
    g["wfo"] = wobj @ wfo                                            # [768,300]
    g["wfc"] = _pad_rows(wfc, 384)                                   # [384,300]
    g["bfeat"] = _pack_bias(bsub @ wfs + bobj @ wfo + bfeat, 3)
    g["wout"] = _pad_rows(np.asarray(params["out"]["w"], np.float32), 384)  # [384,200]
    g["bout"] = _pack_bias(np.asarray(params["out"]["b"], np.float32), 2)
    g["labelT"] = _pad_rows(np.ascontiguousarray(label_emb.T), 256)  # [256,42]

    g["ident"] = np.eye(P, dtype=np.float32)                         # [128,128]
    self_ = np.zeros((1, P), np.float32); self_[0, 0:64] = 1.0
    selb_ = np.zeros((1, P), np.float32); selb_[0, 64:128] = 1.0
    g["self"] = self_
    g["selb"] = selb_
    g["ones_col"] = np.ones((1, P), np.float32)                      # attn bcast lhsT
    g["ones128"] = np.ones((P, 1), np.float32)                       # colsum lhsT
    return g


PARAM_SPECS = {
    # name: (shape, f32r?)
    "wqkv0": ((896, 384), True), "bqkv0": ((3, P), False), "sqkv0": ((3, P), False),
    "wqkv1": ((384, 384), True), "bqkv1": ((3, P), False), "sqkv1": ((3, P), False),
    "wo0": ((P, 300), True), "bo0": ((3, P), False),
    "wo1": ((P, 300), True), "bo1": ((3, P), False),
    "wff1_0": ((384, 600), True), "bff1_0": ((5, P), False),
    "wff1_1": ((384, 600), True), "bff1_1": ((5, P), False),
    "wff2_0": ((640, 300), True), "bff2_0": ((3, P), False),
    "wff2_1": ((640, 300), True), "bff2_1": ((3, P), False),
    "v_la": ((384, 1), True),
    "wfs": ((768, 300), True), "wfo": ((768, 300), True), "wfc": ((384, 300), True),
    "bfeat": ((3, P), False),
    "wout": ((384, 200), True), "bout": ((2, P), False),
    "labelT": ((256, 42), True),
    "ident": ((P, P), True), "self": ((1, P), True), "selb": ((1, P), True),
    "ones_col": ((1, P), True), "ones128": ((P, 1), True),
}


STOP_STAGE = 99


def _build_program():
    nc = bacc.Bacc("TRN2", target_bir_lowering=False, debug=False)
    AF = mybir.ActivationFunctionType
    ALU = mybir.AluOpType

    # ---- DRAM tensors ----
    d_inT = nc.dram_tensor("inT", (EPC, HID, SEQ), F32R, kind="ExternalInput")
    d_wmT = nc.dram_tensor("wmT", (EPC, SEQ, SEQ), U8, kind="ExternalInput")
    d_amT = nc.dram_tensor("amT", (EPC, SEQ, SEQ), U8, kind="ExternalInput")
    d_adj = nc.dram_tensor("adj", (EPC, SEQ, SEQ), U8, kind="ExternalInput")
    d_emb = nc.dram_tensor("embT", (EPC, 60, SEQ), F32R, kind="ExternalInput")
    d_sneg = nc.dram_tensor("sneg", (EPC, SEQ), F32, kind="ExternalInput")
    d_oneg = nc.dram_tensor("oneg", (EPC, SEQ), F32, kind="ExternalInput")
    d_par = {
        n: nc.dram_tensor(n, spec[0], F32R if spec[1] else F32, kind="ExternalInput")
        for n, spec in PARAM_SPECS.items()
    }
    d_sf = nc.dram_tensor("sf", (EPC, 384, SEQ), F32, kind="ExternalOutput")
    d_lg = nc.dram_tensor("lg", (EPC, N_LABELS), F32, kind="ExternalOutput")

    with tile.TileContext(nc) as tc:
        _emit(nc, tc, AF, ALU, d_inT, d_wmT, d_amT, d_adj, d_emb, d_sneg, d_oneg,
              d_par, d_sf, d_lg)
    nc.compile()
    return nc


def _emit(nc, tc, AF, ALU, d_inT, d_wmT, d_amT, d_adj, d_emb, d_sneg, d_oneg,
          d_par, d_sf, d_lg):
    from contextlib import ExitStack
    stack = ExitStack()
    wp = stack.enter_context(tc.tile_pool(name="weights", bufs=1))
    big = stack.enter_context(tc.tile_pool(name="pin", bufs=2))      # h, masks, adf
    pb1 = stack.enter_context(tc.tile_pool(name="pb1", bufs=2))      # qkv, c_t, m_t
    work = stack.enter_context(tc.tile_pool(name="pb2", bufs=2))     # acts
    pbm = stack.enter_context(tc.tile_pool(name="pbm", bufs=1))
    scr = stack.enter_context(tc.tile_pool(name="pscr", bufs=3))     # ex, junk
    prow = stack.enter_context(tc.tile_pool(name="prow", bufs=3))    # [1,512] rows
    pmm = stack.enter_context(tc.tile_pool(name="pmm", bufs=5, space="PSUM"))
    pcp = stack.enter_context(tc.tile_pool(name="pcp", bufs=1, space="PSUM"))
    ppr = stack.enter_context(tc.tile_pool(name="ppr", bufs=1, space="PSUM"))

    # ---- load weights (once) ----
    W = {}
    for name, (shape, _isr) in PARAM_SPECS.items():
        dt = F32R if PARAM_SPECS[name][1] else F32
        if name.startswith(("b", "s")) and name not in ("self", "selb"):
            t = wp.tile([P, shape[0]], dt, tag=name)          # bias/scale: [ntiles,P] -> [P,ntiles]
            nc.sync.dma_start(out=t, in_=d_par[name].ap().rearrange("t p -> p t"))
        elif shape[0] % P == 0 and shape[0] > P:
            kt = shape[0] // P
            t = wp.tile([P, kt, shape[1]], dt, tag=name)
            nc.sync.dma_start(out=t, in_=d_par[name].ap().rearrange("(k p) m -> p k m", p=P))
        else:
            t = wp.tile(list(shape), dt, tag=name)
            nc.sync.dma_start(out=t, in_=d_par[name].ap())
        W[name] = t

    def ktiles(t):
        return t.shape[1] if len(t.shape) == 3 else 1

    def wslice(t, k):
        return t[:, k, :] if len(t.shape) == 3 else t[:, :]

    def mm_proj(wname, rhs_slabs, mslices, bias, scale, func, out_t, extra_bias_stt=None):
        """out_t[:, m, :] = func(scale*(sum_k W[:,k,mcols].T @ rhs_slabs[k]) + bias)."""
        w = W[wname]
        nk = len(rhs_slabs)
        for mi, (m0, m1) in enumerate(mslices):
            ps = pmm.tile([P, SEQ], F32, tag="mm")
            for k in range(nk):
                nc.tensor.matmul(ps[: m1 - m0, :], wslice(w, k)[:, m0:m1],
                                 rhs_slabs[k], start=(k == 0), stop=(k == nk - 1))
            if extra_bias_stt is not None:
                c_t, b_t = extra_bias_stt
                nc.vector.scalar_tensor_tensor(
                    out=out_t[: m1 - m0, mi, :], in0=ps[: m1 - m0, :],
                    scalar=b_t[: m1 - m0, mi : mi + 1], op0=ALU.add,
                    in1=c_t[: m1 - m0, mi, :], op1=ALU.add)
            elif func is None:  # affine only -> DVE
                nc.vector.tensor_scalar(
                    out=out_t[: m1 - m0, mi, :], in0=ps[: m1 - m0, :],
                    scalar1=scale[: m1 - m0, mi : mi + 1], op0=ALU.mult,
                    scalar2=bias[: m1 - m0, mi : mi + 1], op1=ALU.add)
            else:
                sc = scale[: m1 - m0, mi : mi + 1] if scale is not None else 1.0
                bi = bias[: m1 - m0, mi : mi + 1] if bias is not None else 0.0
                nc.scalar.activation(out=out_t[: m1 - m0, mi, :], in_=ps[: m1 - m0, :],
                                     func=func, bias=bi, scale=sc)

    M300 = [(0, 128), (128, 256), (256, 300)]
    M600 = [(0, 128), (128, 256), (256, 384), (384, 512), (512, 600)]

    for e in range(EPC):
        # ---- input DMAs ----
        h = big.tile([P, 7, SEQ], F32R, tag="h")
        nc.sync.dma_start(out=h[:, 0:6, :],
                          in_=d_inT.ap()[e].rearrange("(k p) s -> p k s", p=P))
        nc.vector.memset(h[:, 6, :].bitcast(F32), 0.0)
        nc.sync.dma_start(out=h[0:60, 6, :], in_=d_emb.ap()[e])

        if STOP_STAGE <= 0.2:
            continue
        wm = big.tile([P, 4, SEQ], U8, tag="wm")
        am = big.tile([P, 4, SEQ], U8, tag="am")
        ad = big.tile([P, 4, SEQ], U8, tag="ad")
        for dst, src in ((wm, d_wmT), (am, d_amT), (ad, d_adj)):
            nc.sync.dma_start(out=dst, in_=src.ap()[e].rearrange("(b p) q -> p b q", p=P))

        if STOP_STAGE <= 0.4:
            continue
        # entity mask rows broadcast to all partitions (DMA partition-bcast from DRAM)
        ent = pbm.tile([P, 2, SEQ], F32, tag="ent")
        for i, src in enumerate((d_sneg, d_oneg)):
            bc = bass.AP(src.ap().tensor, e * SEQ, [[0, P], [1, SEQ]])
            nc.sync.dma_start(out=ent[:, i, :], in_=bc)

        if STOP_STAGE <= 0.6:
            continue
        # ---- subj/obj masked max-pool over tokens ----
        pooledf = work.tile([P, 2, 6], F32, tag="pooledf")  # [:,0,:] subj, [:,1,:] obj
        for i in range(2):
            for t in range(6):
                junk = scr.tile([P, SEQ], F32, tag="ex")
                nc.gpsimd.tensor_tensor(
                    out=junk, in0=h[:, t, :].bitcast(F32), in1=ent[:, i, :],
                    op=ALU.add)
                nc.vector.reduce_max(out=pooledf[:, i, t : t + 1], in_=junk,
                                     axis=mybir.AxisListType.X)

        pooled = work.tile([P, 2, 6], F32R, tag="pooled")
        nc.vector.tensor_copy(out=pooled, in_=pooledf)
        if STOP_STAGE <= 1:
            continue
        # ---- 2 DGA layers ----
        hin_slabs = [h[:, k, :] for k in range(7)]
        for li in range(2):
            qkv = pb1.tile([P, 3, SEQ], F32R, tag="qkv")
            mm_proj(f"wqkv{li}", hin_slabs, [(0, 128), (128, 256), (256, 384)],
                    W[f"bqkv{li}"], W[f"sqkv{li}"], None, qkv)
            QK = {
                "f": (qkv[0:64, 0, :], qkv[0:64, 1, :], qkv[0:64, 2, :], wm, (0, 64)),
                "b": (qkv[64:128, 0, :], qkv[64:128, 1, :], qkv[64:128, 2, :], am, (64, 128)),
            }
            cps = {}
            sums = {}
            for di, d in enumerate(("f", "b")):
                Q, K, V, msk, (i0, i1) = QK[d]
                vau = work.tile([P, 4, 65], F32R, tag=f"vau{d}")
                nc.vector.memset(vau[:, :, 64:65].bitcast(F32), 1.0)
                for kb in range(4):
                    vps = pmm.tile([P, 65], F32R, tag="mm")
                    nc.tensor.transpose(vps[:, 0:64], V[:, kb * P : (kb + 1) * P],
                                        W["ident"][i0:i1, i0:i1])
                    nc.vector.tensor_copy(out=vau[:, kb, 0:64], in_=vps[:, 0:64])
                aun = work.tile([P, 4, SEQ], F32R, tag="aun")
                for kb in range(4):
                    sps = pmm.tile([P, SEQ], F32, tag="mm")
                    nc.tensor.matmul(sps, K[:, kb * P : (kb + 1) * P], Q,
                                     start=True, stop=True)
                    ex = scr.tile([P, SEQ], F32, tag="ex")
                    nc.scalar.activation(out=ex, in_=sps, func=AF.Exp)
                    nc.gpsimd.tensor_tensor(out=aun[:, kb, :], in0=ex,
                                            in1=msk[:, kb, :], op=ALU.mult)
                cp = pcp.tile([P, SEQ], F32, tag=f"cp{d}")
                for kb in range(4):
                    nc.tensor.matmul(cp[0:65, :], vau[:, kb, :], aun[:, kb, :],
                                     start=(kb == 0), stop=(kb == 3))
                cps[d] = cp
                sr = prow.tile([1, SEQ], F32R, tag="rows", name=f"sums_{d}")
                nc.vector.tensor_copy(out=sr, in_=cp[64:65, :])
                sums[d] = sr
            ccat = work.tile([P, SEQ], F32R, tag="ccat")
            rec = work.tile([P, SEQ], F32, tag="rec")
            for di, d in enumerate(("f", "b")):
                rp = pmm.tile([P, SEQ], F32, tag="mm", name=f"rp_{d}")
                nc.tensor.matmul(rp[0:64, :], W["ones_col"][:, 0:64], sums[d],
                                 start=True, stop=True)
                r0, r1 = (0, 64) if di == 0 else (64, 128)
                nc.vector.reciprocal(out=rec[r0:r1, :], in_=rp[0:64, :])
                nc.vector.tensor_tensor(out=ccat[r0:r1, :], in0=cps[d][0:64, :],
                                        in1=rec[r0:r1, :], op=ALU.mult)

            c_t = pb1.tile([P, 3, SEQ], F32R, tag="c_t")
            nc.vector.memset(c_t[:, 2, :].bitcast(F32), 0.0)
            wo = W[f"wo{li}"]
            for mi, (m0, m1) in enumerate(M300):
                pso = pmm.tile([P, SEQ], F32, tag="mm", name="pso")
                nc.tensor.matmul(pso[: m1 - m0, :], wo[0:64, m0:m1], ccat[0:64, :],
                                 start=True, stop=False)
                nc.tensor.matmul(pso[: m1 - m0, :], wo[64:128, m0:m1], ccat[64:128, :],
                                 start=False, stop=True)
                nc.scalar.activation(out=c_t[: m1 - m0, mi, :], in_=pso[: m1 - m0, :],
                                     func=AF.Gelu, bias=W[f"bo{li}"][: m1 - m0, mi : mi + 1])
            m_t = pbm.tile([P, 5, SEQ], F32R, tag="m_t")
            nc.vector.memset(m_t[:, 4, :].bitcast(F32), 0.0)
            mm_proj(f"wff1_{li}", [c_t[:, k, :] for k in range(3)], M600,
                    W[f"bff1_{li}"], None, AF.Gelu, m_t)
            h2 = work.tile([P, 3, SEQ], F32R, tag="h2")
            nc.vector.memset(h2[:, 2, :].bitcast(F32), 0.0)
            mm_proj(f"wff2_{li}", [m_t[:, k, :] for k in range(5)], M300,
                    None, None, None, h2, extra_bias_stt=(c_t, W[f"bff2_{li}"]))
            hin_slabs = [h2[:, k, :] for k in range(3)]

        if STOP_STAGE <= 2:
            continue
        # ---- structure_feature out ----
        nc.sync.dma_start(out=d_sf.ap()[e, 0:256, :].rearrange("(k p) s -> p k s", p=P),
                          in_=h2[:, 0:2, :].bitcast(F32))
        nc.sync.dma_start(out=d_sf.ap()[e, 256:300, :], in_=h2[0:44, 2, :].bitcast(F32))

        # ---- pool mask: degree = rowsum + colsum of adj ----
        rs = prow.tile([P, 4], F32, tag="small")
        csps = ppr.tile([1, SEQ], F32, tag="pr")
        for qb in range(4):
            adf = big.tile([P, SEQ], F32R, tag="adf")
            eng = nc.gpsimd if qb % 2 == 0 else nc.vector
            eng.tensor_copy(out=adf, in_=ad[:, qb, :])
            nc.vector.reduce_sum(out=rs[:, qb : qb + 1], in_=adf,
                                 axis=mybir.AxisListType.X)
            nc.tensor.matmul(csps, W["ones128"], adf,
                             start=(qb == 0), stop=(qb == 3))
        rtps = pcp.tile([1, SEQ], F32R, tag="cpf")
        rsr = prow.tile([P, 4], F32R, tag="small")
        nc.vector.tensor_copy(out=rsr, in_=rs)
        for qb in range(4):
            nc.tensor.transpose(rtps[0:1, qb * P : (qb + 1) * P], rsr[:, qb : qb + 1],
                                W["ident"])
        cs_sb = prow.tile([1, SEQ], F32, tag="rows")
        nc.vector.tensor_copy(out=cs_sb, in_=csps)
        deg = prow.tile([1, SEQ], F32, tag="rows")
        nc.vector.tensor_tensor(out=deg, in0=cs_sb, in1=rtps, op=ALU.add)
        pneg = prow.tile([1, SEQ], F32, tag="rows")
        nc.vector.tensor_scalar(out=pneg, in0=deg, scalar1=0.0, scalar2=NEG,
                                op0=ALU.is_equal, op1=ALU.mult)

        if STOP_STAGE <= 3:
            continue
        # ---- label attention scores over seq ----
        sps2 = ppr.tile([1, SEQ], F32, tag="pr")
        for k in range(3):
            nc.tensor.matmul(sps2, wslice(W["v_la"], k), hin_slabs[k],
                             start=(k == 0), stop=(k == 2))
        sco = prow.tile([1, SEQ], F32, tag="rows")
        nc.vector.tensor_tensor(out=sco, in0=pneg, in1=sps2, op=ALU.add)
        att = prow.tile([1, SEQ], F32, tag="rows")
        asum = prow.tile([1, 1], F32, tag="small")
        nc.scalar.activation(out=att, in_=sco, func=AF.Exp, accum_out=asum)
        rsum = prow.tile([1, 1], F32, tag="small")
        nc.vector.reciprocal(out=rsum, in_=asum)
        attn = prow.tile([1, SEQ], F32R, tag="rows")
        nc.vector.tensor_scalar(out=attn, in0=att, scalar1=rsum, scalar2=None, op0=ALU.mult)
        abps = pmm.tile([P, SEQ], F32, tag="mm")
        nc.tensor.matmul(abps, W["ones_col"], attn, start=True, stop=True)
        ctxf = prow.tile([P, 3], F32, tag="small")
        for k in range(3):
            junk = scr.tile([P, SEQ], F32, tag="ex")
            nc.vector.tensor_tensor(
                out=junk, in0=hin_slabs[k].bitcast(F32), in1=abps, op=ALU.mult)
            nc.vector.reduce_sum(out=ctxf[:, k : k + 1], in_=junk,
                                 axis=mybir.AxisListType.X)
        ctx = work.tile([P, 3], F32R, tag="ctx")
        nc.vector.tensor_copy(out=ctx, in_=ctxf)

        if STOP_STAGE <= 4:
            continue
        # ---- head: features -> out200 -> logits ----
        feat = work.tile([P, 3, 1], F32R, tag="feat")
        nc.vector.memset(feat.bitcast(F32), 0.0)
        for mi, (m0, m1) in enumerate(M300):
            ps = ppr.tile([P, 1], F32, tag="pr")
            nmm = 15
            i = 0
            for k in range(6):
                nc.tensor.matmul(ps[: m1 - m0, :], W["wfs"][:, k, m0:m1].bitcast(F32),
                                 pooled[:, 0, k : k + 1].bitcast(F32), start=(i == 0), stop=(i == nmm - 1)); i += 1
            for k in range(6):
                nc.tensor.matmul(ps[: m1 - m0, :], W["wfo"][:, k, m0:m1].bitcast(F32),
                                 pooled[:, 1, k : k + 1].bitcast(F32), start=(i == 0), stop=(i == nmm - 1)); i += 1
            for k in range(3):
                nc.tensor.matmul(ps[: m1 - m0, :], W["wfc"][:, k, m0:m1].bitcast(F32),
                                 ctx[:, k : k + 1].bitcast(F32), start=(i == 0), stop=(i == nmm - 1)); i += 1
            nc.scalar.activation(out=feat[: m1 - m0, mi, :], in_=ps[: m1 - m0, :],
                                 func=AF.Gelu, bias=W["bfeat"][: m1 - m0, mi : mi + 1])
        o200 = work.tile([P, 2, 1], F32R, tag="o200")
        nc.vector.memset(o200.bitcast(F32), 0.0)
        for mi, (m0, m1) in enumerate([(0, 128), (128, 200)]):
            ps = ppr.tile([P, 1], F32, tag="pr")
            for k in range(3):
                nc.tensor.matmul(ps[: m1 - m0, :], W["wout"][:, k, m0:m1].bitcast(F32),
                                 feat[:, k, :].bitcast(F32), start=(k == 0), stop=(k == 2))
            nc.scalar.activation(out=o200[: m1 - m0, mi, :], in_=ps[: m1 - m0, :],
                                 func=AF.Identity, bias=W["bout"][: m1 - m0, mi : mi + 1])
        lps = ppr.tile([N_LABELS, 1], F32, tag="pr")
        for k in range(2):
            nc.tensor.matmul(lps, W["labelT"][:, k, :].bitcast(F32),
                             o200[:, k, :].bitcast(F32), start=(k == 0), stop=(k == 1))
        lg_sb = prow.tile([N_LABELS, 1], F32, tag="small")
        nc.scalar.activation(out=lg_sb, in_=lps, func=AF.Copy)
        nc.sync.dma_start(out=d_lg.ap()[e : e + 1, :], in_=lg_sb)
    stack.close()


_CACHED = {}


def _get_program():
    if "nc" not in _CACHED:
        _CACHED["nc"] = _build_program()
    return _CACHED["nc"]


def host_prep(inputs, pos_ids, subj_pos, obj_pos, dist, adj, whole_adj,
              ancestor_adj, input_mask, params):
    inputs = np.asarray(inputs, np.float32)
    g = _prep_params(params)
    pos_emb = np.asarray(params["pos_emb"], np.float32)
    dist_emb = np.asarray(params["dist_emb"], np.float32)

    # host marshalling (layout only)
    inT = np.ascontiguousarray(inputs.transpose(0, 2, 1))            # [32,768,512]
    wmT = np.ascontiguousarray(np.asarray(whole_adj).transpose(0, 2, 1)).astype(np.uint8)
    amT = np.ascontiguousarray(np.asarray(ancestor_adj).transpose(0, 2, 1)).astype(np.uint8)
    adj8 = np.ascontiguousarray(np.asarray(adj)).astype(np.uint8)
    embT = np.concatenate(
        [pos_emb[np.asarray(pos_ids)], dist_emb[np.asarray(dist)]], axis=2
    ).transpose(0, 2, 1)                                             # [32,60,512]
    embT = np.ascontiguousarray(embT, np.float32)
    sneg = ((np.asarray(subj_pos) != POS_IND) * np.float32(-1e30)).astype(np.float32)
    oneg = ((np.asarray(obj_pos) != POS_IND) * np.float32(-1e30)).astype(np.float32)

    in_maps = []
    for c in range(NCORES):
        s = slice(c * EPC, (c + 1) * EPC)
        m = {"inT": inT[s], "wmT": wmT[s], "amT": amT[s], "adj": adj8[s],
             "embT": embT[s], "sneg": sneg[s], "oneg": oneg[s]}
        m.update(g)
        in_maps.append(m)
    return in_maps


def kernel(**inputs):
    in_maps = host_prep(**inputs)
    nc = _get_program()
    res = bass_utils.run_bass_kernel_spmd(nc, in_maps, core_ids=list(range(NCORES)))

    logits = np.concatenate([r["lg"] for r in res.results], axis=0)  # [32,42]
    sf = np.concatenate([r["sf"] for r in res.results], axis=0)      # [32,384,512]
    structure = np.ascontiguousarray(sf[:, :300, :].transpose(0, 2, 1))
    return logits, structure
